# revision 1
# baseline (speedup 1.0000x reference)
"""Capsule-routing kernel for Trainium2 (8 NeuronCores, data-parallel over batch).

Reference (per item, S=512 input caps, N=32 output caps, D=64, 3 iters):
    u_hat = (u @ W).reshape(S, N, D)        # never materialized
    b = 0
    for it in 0..2:
        c = softmax(b, axis=caps)
        o = squash(einsum('ns,nsd->nd', c, u_hat))   # squash = L2 normalize
        if it < 2: b = einsum('nd,nsd->ns', o, u_hat)

Re-association (per item):
    mT[i, n] = sum_s u[s,i] c[n,s]            (m-step, contract s)
    o[n, d]  = sum_i mT[i,n] W[i, n*64+d]     (o-step, block-diag, contract i)
    P[i, n]  = sum_d W[i, n*64+d] o[n,d]      (P-step, contract d via W^T)
    r[n, s]  = sum_i P[i,n] u[s,i]            (b-step, contract i via u^T)

Everything runs in bf16 on the PE (fp32 PSUM accumulate). Matmuls are
oriented so the output free dim (what the PE costs) is minimal: m/b-steps
put i/s on partitions (out free = 32), o/P-steps batch a 4-item group in
the free dim (out free = 4/8).

Key mechanisms:
  - u and W are cast-loaded fp32->bf16 straight from DRAM by SWDGE (gpsimd)
    DMAs -- halves DMA-engine time vs fp32 loads and eliminates all on-chip
    convert traffic.
  - s is laid out s = 4p + j (p = partition, j = 0..3) so each item's u
    loads as one DMA with 8 KB contiguous runs.
  - u^T via PE transposes (bf16 stays bf16 in PSUM -> 2x-rate copies out).
  - Squash: transpose o to (n,b)-on-partitions, Square with accum_out
    (fused row-sum), rsqrt = exp(-0.5*ln(x+eps)) to stay in one activation
    table, scale on DVE.
  - PSUM is 8 banks; every pipeline stage packs its whole 4-item group into
    one bank so softmax/copies run as single wide instructions.

Batch 64 -> 8 items/core; routing pipelined in 2 groups of 4 items.
"""

import sys

import numpy as np

if "/opt/trn_rl_repo" not in sys.path:
    sys.path.insert(0, "/opt/trn_rl_repo")

import concourse.bass as bass  # noqa: F401
import concourse.mybir as mybir
import concourse.tile as tile
from concourse import bacc
from concourse.masks import make_identity

# Keep Exp/Ln/Square/Copy resolvable via one activation table so the kernel
# needs a single LoadActFuncSet (table swaps cost ~1.3us each).
_orig_get_tables = bacc.get_activation_tables


def _tables_prefer_nle(arch):
    t = _orig_get_tables(arch)
    pref = "natural_log_exp_and_others"
    if pref not in t:
        return t
    mine = t[pref]
    return {k: (v if k == pref else v - mine) for k, v in t.items()}


bacc.get_activation_tables = _tables_prefer_nle

FP = mybir.dt.float32
BF = mybir.dt.bfloat16
EPS = 1e-7
B, S, I = 64, 512, 512          # full batch, input caps, input dim
N, D = 32, 64                   # output caps, cap dim
NCORES = 8
BC = B // NCORES                # items per core = 8
G = 2                           # routing groups
BG = BC // G                    # items per group = 4
P = 128
IC = I // P                     # i chunks = 4
J = 4                           # s = 4*p + j
ROUTINGS = 3
ND = N * D


def _ap(base, offset_delta, dims):
    return bass.AP(tensor=base.tensor, offset=base.offset + offset_delta,
                   ap=dims)


def _copy(eng, nc, dst, src):
    if eng is nc.scalar:
        eng.copy(dst, src)
    else:
        eng.tensor_copy(dst, src)


def build_kernel(nc):
    u_dram = nc.dram_tensor("u", [BC, S, I], FP, kind="ExternalInput").ap()
    w_dram = nc.dram_tensor("W", [I, ND], FP, kind="ExternalInput").ap()
    o_dram = nc.dram_tensor("out", [BC, N, D], FP, kind="ExternalOutput").ap()

    with tile.TileContext(nc) as tc:
        _body(tc, u_dram, w_dram, o_dram)
    return nc


def _body(tc, u_dram, w_dram, o_dram):
    from contextlib import ExitStack

    nc = tc.nc
    ctx = ExitStack()
    with ctx:
        statics = ctx.enter_context(tc.tile_pool(name="statics", bufs=1))
        stage = ctx.enter_context(tc.tile_pool(name="stage", bufs=3))
        psum = ctx.enter_context(tc.tile_pool(name="psum", bufs=2, space="PSUM"))

        # ---------- statics ----------
        ident_f = statics.tile([P, P], FP)
        make_identity(nc, ident_f)
        ident = statics.tile([P, P], BF)
        nc.vector.tensor_copy(ident, ident_f)
        eps_sb = statics.tile([P, 1], FP)
        nc.vector.memset(eps_sb, EPS)

        w_bf = statics.tile([P, IC, ND], BF)     # W[128*ic+p, nd]
        wt = statics.tile([P, N // 2, I], BF)    # W[i, 128*q+p] at [p, q, i]
        u_bf = statics.tile([P, BC, J, I], BF)   # u[b, 4p+j, i]
        ut = statics.tile([P, BC, J * IC, P], BF)  # u[b,4q+j,128*ic+v] at [v,b,4j+ic,q]
        ct = statics.tile([P, BC, J, N], BF)     # c[b, n, 4p+j]
        mt = statics.tile([P, IC, N, BC], BF)    # m[b, n, 128*ic+v] at [v, ic, n, b]
        bd = [statics.tile([P, N // 2, 2 * BG], BF, name=f"bd{g}")
              for g in range(G)]                 # o[4g+bi, 2q+h, d] at [64h+d, q, 4h+bi]

        ones_n = statics.tile([P, 1], BF)
        nc.vector.memset(ones_n, 1.0 / N)
        for g in range(G):
            nc.vector.memset(bd[g].rearrange("p a b -> p (a b)"), 0.0)

        # ---------- cast-loads (SWDGE converts fp32 -> bf16 in the DMA) ----------
        # Only 8 SWDGE completion sems exist; a 9th SWDGE DMA stalls until an
        # earlier sem's waiters all retire. Load u in 2-item chunks so W + 4
        # chunks = 5 SWDGE DMAs.
        with tc.high_priority(offset=-2000):
            nc.gpsimd.dma_start(
                out=w_bf, in_=w_dram.rearrange("(c p) n -> p c n", p=P))
            for ch in range(BC // 2):
                nc.gpsimd.dma_start(
                    out=u_bf[:, 2 * ch:2 * ch + 2],
                    in_=u_dram[2 * ch:2 * ch + 2].rearrange(
                        "b (p j) i -> p b j i", j=J))
                if ch == 2:
                    nc.sync.dma_start(out=ut[:, 4], in_=u_bf[:, 4],
                                      transpose=True)
                    nc.sync.dma_start(out=ut[:, 5], in_=u_bf[:, 5],
                                      transpose=True)
            nc.sync.dma_start(out=ut[:, 6], in_=u_bf[:, 6], transpose=True)
            nc.sync.dma_start(out=ut[:, 7], in_=u_bf[:, 7], transpose=True)

        # ---------- W^T build (PE transposes; feeds P-step) ----------
        bld_prio = tc.high_priority(offset=-1000)
        bld_prio.__enter__()
        for qq in range(N // 4):
            tbw = psum.tile([P, 1024], BF, tag="tp", name="tbw", bufs=3)
            for dq in range(2):
                q = 2 * qq + dq
                for ic in range(IC):
                    nc.tensor.transpose(
                        tbw[:, (dq * IC + ic) * P:(dq * IC + ic + 1) * P],
                        w_bf[:, ic, q * P:(q + 1) * P], ident)
            nc.scalar.copy(wt[:, 2 * qq:2 * qq + 2, :], tbw)

        # ---------- u^T build ----------
        # Early items via PE transposes + DVE/Act copies (those engines are
        # still idle); late items via the DMA crossbar transpose, which the
        # (otherwise idle after the loads) DMA engines finish right when the
        # second group's b-step needs them.
        N_PE_UT = 4
        for b in range(N_PE_UT):
            for half in range(2):
                tbu = psum.tile([P, 1024], BF, tag="tp", name="tbu", bufs=3)
                for jj in range(2):
                    j = 2 * half + jj
                    for ic in range(IC):
                        nc.tensor.transpose(
                            tbu[:, (jj * IC + ic) * P:(jj * IC + ic + 1) * P],
                            u_bf[:, b, j, ic * P:(ic + 1) * P], ident)
                nc.vector.tensor_copy(
                    ut[:, b, half * 2 * IC:(half + 1) * 2 * IC, :], tbu)
        # (ut4-7 DMA transposes are issued inside the load stream above)
        bld_prio.__exit__(None, None, None)

        # PE p-state warmers before the final iteration: junk transposes in
        # the natural PE idle window so it2's matmuls run at full clock.
        junk = psum.tile([P, 1024], BF, tag="tp", name="junk", bufs=3)

        def fill(k):
            for _ in range(k):
                nc.tensor.transpose(junk[:, 0:P], ident, ident)

        # ---------- routing ----------
        for it in range(ROUTINGS):
            for g in range(G):
                if it == ROUTINGS - 1:
                    fill(20 if g == 0 else 26)
                _route_iter(tc, stage, psum, o_dram, it, g, ident, eps_sb,
                            w_bf, wt, u_bf, ut, ct, mt, bd, ones_n)


def _route_iter(tc, stage, psum, o_dram, it, g, ident, eps_sb,
                w_bf, wt, u_bf, ut, ct, mt, bd, ones_n):
    nc = tc.nc
    last = it == ROUTINGS - 1

    # m-step: mT[v, ic, n] per item; contract s = (p, j) on partitions.
    # One PSUM bank holds the whole group's m.
    pm = psum.tile([P, BG, IC, N], FP, tag="pm", name="pm", bufs=1)
    if it == 0:
        # c == 1/N exactly at iter 0: m0[i] = (1/N) sum_s u[s,i] is the same
        # for every cap -> one column per (item, i-chunk) via a ones-vector.
        for bi in range(BG):
            b = BG * g + bi
            for ic in range(IC):
                for j in range(J):
                    nc.tensor.matmul(
                        pm[:, bi, ic, 0:1],
                        lhsT=u_bf[:, b, j, ic * P:(ic + 1) * P],
                        rhs=ones_n,
                        start=(j == 0), stop=(j == J - 1))
        nc.vector.tensor_copy(
            mt[:, :, 0, BG * g:BG * (g + 1)],
            pm[:, :, :, 0].rearrange("p b i -> p i b"))
    else:
        for bi in range(BG):
            b = BG * g + bi
            for ic in range(IC):
                for j in range(J):
                    nc.tensor.matmul(
                        pm[:, bi, ic, :],
                        lhsT=u_bf[:, b, j, ic * P:(ic + 1) * P],
                        rhs=ct[:, b, j, :],
                        start=(j == 0), stop=(j == J - 1))
        nc.vector.tensor_copy(mt[:, :, :, BG * g:BG * (g + 1)],
                              pm.rearrange("p b i n -> p i n b"))

    # squash bank: ot (fp32, o-step dst) | on (bf16) | ots (bf16), carved
    # from one 2KB PSUM bank.
    sqb = psum.tile([P, 1024], BF, tag="sq", name="sqb", bufs=2)
    ot = sqb.bitcast(FP)[0:D, 0:N * BG]
    on_ps = sqb[:, 256:256 + D]
    oTs = sqb[0:D, 384:384 + P]

    # o-step: ot[d, 4n+bi] = sum_i mT[i,n] W[i, n*64+d] for the group
    for n in range(N):
        for ic in range(IC):
            n_src = 0 if it == 0 else n
            nc.tensor.matmul(
                ot[:, n * BG:(n + 1) * BG],
                lhsT=w_bf[:, ic, n * D:(n + 1) * D],
                rhs=mt[:, ic, n_src, BG * g:BG * (g + 1)],
                start=(ic == 0), stop=(ic == IC - 1))

    # squash: transpose to (n,bi)-on-partitions, normalize rows
    oTu = stage.tile([D, N * BG], BF, tag="oTu", name="oTu")
    nc.scalar.copy(oTu, ot)
    nc.tensor.transpose(on_ps, oTu, ident[:D, :D])
    # sqs is write-only scratch (only accum_out matters); park it in the
    # free upper half of the squash PSUM bank so the Act op pays the cheaper
    # PSUM access init on both operands.
    sqs = sqb.bitcast(FP)[:, 256:256 + D]
    n2 = stage.tile([P, 1], FP, tag="n2", name="n2")
    nc.scalar.activation(sqs, on_ps, mybir.ActivationFunctionType.Square,
                         accum_out=n2)
    lg = stage.tile([P, 1], FP, tag="lg", name="lg")
    nc.scalar.activation(lg, n2, mybir.ActivationFunctionType.Ln,
                         bias=eps_sb[:, 0:1])
    rs = stage.tile([P, 1], FP, tag="rs", name="rs")
    nc.scalar.activation(rs, lg, mybir.ActivationFunctionType.Exp, scale=-0.5)
    rs_b = bass.AP(tensor=rs.tensor, offset=rs.offset, ap=[rs.ap[0], [0, D]])

    if last:
        onf = stage.tile([P, D], FP, tag="onf", name="onf")
        nc.scalar.activation(onf, on_ps, mybir.ActivationFunctionType.Copy,
                             scale=rs[:, 0:1])
        dst = _ap(o_dram, BG * g * N * D, [[D, N], [N * D, BG], [1, D]])
        nc.sync.dma_start(out=dst, in_=onf)
        return

    onb = stage.tile([P, D], BF, tag="onb", name="onb")
    nc.scalar.activation(onb, on_ps, mybir.ActivationFunctionType.Copy,
                         scale=rs[:, 0:1])

    # block-diag o^T for the P-step: bd[64h+d, q, 4h+bi] = o[4g+bi, 2q+h, d]
    nc.tensor.transpose(oTs, onb, ident)
    oTs_v = oTs.rearrange("p (q x) -> p q x", q=N // 2)
    nc.vector.tensor_copy(bd[g][0:D, :, 0:BG], oTs_v[:, :, 0:BG])
    nc.vector.tensor_copy(bd[g][D:P, :, BG:2 * BG], oTs_v[:, :, BG:2 * BG])

    # P-step: P[i, (h,bi)] per pair q; contract (h,d) on partitions
    pp = psum.tile([P, IC, N // 2, 2 * BG], FP, tag="pp", name="pp", bufs=1)
    for ic in range(IC):
        for q in range(N // 2):
            nc.tensor.matmul(
                pp[:, ic, q, :],
                lhsT=wt[:, q, ic * P:(ic + 1) * P],
                rhs=bd[g][:, q, :],
                start=True, stop=True)
    pt = stage.tile([P, IC, N // 2, 2 * BG], BF, tag="pt", name="pt")
    nc.scalar.copy(pt, pp)

    # b-step + per-item softmax (so the next iteration's m-step for item bi
    # starts as soon as its own softmax lands, not the whole group's)
    rt = psum.tile([P, BG, J, N], FP, tag="rt", name="rt", bufs=1)
    for bi in range(BG):
        b = BG * g + bi
        for j in range(J):
            for ic in range(IC):
                rhs = _ap(pt, ic * (N // 2) * 2 * BG + bi,
                          [pt.ap[0], [2 * BG, N // 2], [BG, 2]])
                nc.tensor.matmul(
                    rt[:, bi, j, :],
                    lhsT=ut[:, b, j * IC + ic, :],
                    rhs=rhs,
                    start=(ic == 0), stop=(ic == IC - 1))
        if bi % 2 == 1:
            b0 = b - 1
            et = stage.tile([P, 2 * J, N], FP, tag="et", name=f"et{bi}",
                            bufs=4)
            nc.scalar.activation(
                et, rt[:, bi - 1:bi + 1].rearrange("p b j n -> p (b j) n"),
                mybir.ActivationFunctionType.Exp)
            zz = stage.tile([P, 2 * J], FP, tag="zz", name=f"zz{bi}", bufs=4)
            nc.vector.reduce_sum(zz, et, axis=mybir.AxisListType.X)
            rz = stage.tile([P, 2 * J], FP, tag="rz", name=f"rz{bi}", bufs=4)
            nc.vector.reciprocal(rz, zz)
            rz_b = bass.AP(tensor=rz.tensor, offset=rz.offset,
                           ap=[rz.ap[0], [1, 2 * J], [0, N]])
            nc.vector.tensor_tensor(
                ct[:, b0:b0 + 2].rearrange("p b j n -> p (b j) n"),
                et, rz_b, mybir.AluOpType.mult)


_COMPILED = None


def _get_compiled():
    global _COMPILED
    if _COMPILED is None:
        nc = bacc.Bacc("TRN2", target_bir_lowering=False, debug=False,
                       num_devices=NCORES,
                       dynamic_dma_scratch_size=49152)
        build_kernel(nc)
        nc.compile()
        _COMPILED = nc
    return _COMPILED


def kernel(u_vecs, W):
    from concourse.bass_utils import run_bass_kernel_spmd

    u_vecs = np.ascontiguousarray(u_vecs, dtype=np.float32)
    W = np.ascontiguousarray(W, dtype=np.float32)
    assert u_vecs.shape == (B, S, I) and W.shape == (I, ND)

    nc = _get_compiled()
    in_maps = [
        {"u": u_vecs[c * BC:(c + 1) * BC], "W": W} for c in range(NCORES)
    ]
    res = run_bass_kernel_spmd(nc, in_maps, list(range(NCORES)))
    return np.concatenate(
        [res.results[c]["out"] for c in range(NCORES)], axis=0
    ).astype(np.float32)

